# revision 1
# baseline (speedup 1.0000x reference)
"""HR2O_NL sparse-attention kernel for 8 Trainium2 NeuronCores.

Sharding: data-parallel over ROI groups (videos LPT-binpacked onto 8 cores,
whole groups stay local). Conv weights + GN params replicated. Each core runs
q/k/v 3x3 convs (bf16 matmuls, fp32 PSUM), per-position masked attention,
GroupNorm, relu, out-conv, residual — all on its ROI shard (padded to CAP=70).
"""
import sys, types
import numpy as np
import ml_dtypes

import concourse.bass as bass
import concourse.mybir as mybir
import concourse.tile as tile
from concourse.bass_utils import run_bass_kernel_spmd

BF = mybir.dt.bfloat16
F32 = mybir.dt.float32
CAP = 70          # padded ROIs per core
NB = 7            # roi blocks of 10
C = 512
P = 49            # 7x7 positions
NCORE = 8
NPOS = CAP * P    # 3430


def _install_profhook():
    if 'antenv.axon_hooks' in sys.modules:
        return
    try:
        from trn_agent_boot.trn_boot import _ntff_profile_via_ctypes
        hook = _ntff_profile_via_ctypes('/opt/axon/libaxon_pjrt.so')
    except Exception:
        hook = None
    m = types.ModuleType('antenv.axon_hooks')
    m.get_axon_ntff_profile_hook = lambda: hook
    sys.modules['antenv.axon_hooks'] = m


def _walk_blocks(bb):
    yield bb
    for inner in getattr(bb, 'blocks', []) or []:
        yield from _walk_blocks(inner)


def _split_multiwait(nc):
    # this walrus build accepts one sync wait per instruction
    fn = nc.m.functions[0]
    for bb in list(_walk_blocks(fn)):
        insts = getattr(bb, 'instructions', None)
        if not insts:
            continue
        new_list, changed = [], False
        for inst in insts:
            si = inst.sync_info
            if si is not None and si.on_wait is not None and len(si.on_wait) > 1:
                waits = list(si.on_wait)
                for j, w in enumerate(waits[:-1]):
                    d = mybir.InstDrain(name=f"{inst.name}_ws{j}", ins=[], outs=[])
                    d.engine = inst.engine
                    d.sync_info = mybir.SyncInfo(on_wait=[w], on_update=[])
                    new_list.append(d)
                si.on_wait = [waits[-1]]
                changed = True
            new_list.append(inst)
        if changed:
            insts[:] = new_list


_NC_CACHE = {}


def _build():
    if 'nc' in _NC_CACHE:
        return _NC_CACHE['nc']
    nc = bass.Bass("TRN2", target_bir_lowering=False, debug=False, num_devices=NCORE)
    xp_d = nc.dram_tensor("xp", [4, 128, CAP * 81], BF, kind="ExternalInput")
    xint_d = nc.dram_tensor("xint", [4, 128, NPOS], F32, kind="ExternalInput")
    wq_d = nc.dram_tensor("wq", [4, 128, 9, 4, 128], BF, kind="ExternalInput")
    wk_d = nc.dram_tensor("wk", [4, 128, 9, 4, 128], BF, kind="ExternalInput")
    wv_d = nc.dram_tensor("wv", [4, 128, 9, 4, 128], BF, kind="ExternalInput")
    wo_d = nc.dram_tensor("wo", [4, 128, 9, 4, 128], BF, kind="ExternalInput")
    mask_d = nc.dram_tensor("mask", [CAP, CAP], F32, kind="ExternalInput")
    y_d = nc.dram_tensor("y", [4, 128, NPOS], F32, kind="ExternalOutput")
    v_dram = nc.dram_tensor("v_sc", [CAP, 4, 128, P], BF)
    sc1 = nc.dram_tensor("sc1", [NPOS], F32)   # recip bounce
    sc2 = nc.dram_tensor("sc2", [NPOS], F32)   # rstd bounce
    sc3 = nc.dram_tensor("sc3", [NPOS], F32)   # negB bounce

    def conv_rhs(xt, blk, tap):
        dy, dx = tap // 3, tap % 3
        return bass.AP(tensor=xt.tensor, offset=xt.offset + blk * 810 + dy * 9 + dx,
                       ap=[xt.ap[0], [81, 10], [9, 7], [1, 7]])

    def bcast_read(handle, n):
        base = handle[:]
        return bass.AP(tensor=base.tensor, offset=0, ap=[[0, 128], [1, n]])

    with tile.TileContext(nc) as tc:
        with tc.tile_pool(name="persist", bufs=1) as pp:
            attw = pp.tile([70, P, 70], BF, name="attw")
            virt = [pp.tile([128, P, 70], F32, name=f"virt{t}") for t in range(4)]
            qkp_cm = tc.tile_pool(name="qk", bufs=1)
            qkp = qkp_cm.__enter__()
            q_s = [qkp.tile([128, NPOS], BF, name=f"q{t}") for t in range(4)]
            k_s = [qkp.tile([128, NPOS], BF, name=f"k{t}") for t in range(4)]

            # ---------------- phase 1: q,k,v convs ----------------
            with (
                tc.tile_pool(name="p1", bufs=1) as p1,
                tc.tile_pool(name="wts", bufs=2) as wts,
                tc.tile_pool(name="vst", bufs=3) as vst,
                tc.tile_pool(name="ps1", bufs=8, space="PSUM") as ps1,
            ):
                xt = [p1.tile([128, CAP * 81], BF, name=f"xp{c}") for c in range(4)]
                for c in range(4):
                    nc.sync.dma_start(out=xt[c][:], in_=xp_d[c])
                for wd, dst in ((wq_d, q_s), (wk_d, k_s), (wv_d, None)):
                    for cto in range(4):
                        wt = wts.tile([128, 4, 9, 128], BF, name="wt", tag="wt")
                        for ci in range(4):
                            srcap = bass.AP(
                                tensor=wd[:].tensor, offset=ci * 589824 + cto * 128,
                                ap=[[4608, 128], [512, 9], [1, 128]])
                            nc.sync.dma_start(out=wt[:, ci, :, :], in_=srcap)
                        for blk in range(NB):
                            acc = ps1.tile([128, 490], F32, name="acc", tag="acc")
                            fi = True
                            for ci in range(4):
                                for tap in range(9):
                                    nc.tensor.matmul(acc[:], wt[:, ci, tap, :],
                                                     conv_rhs(xt[ci], blk, tap),
                                                     start=fi, stop=(ci == 3 and tap == 8))
                                    fi = False
                            if dst is not None:
                                nc.vector.tensor_copy(
                                    dst[cto][:, blk * 490:(blk + 1) * 490], acc[:])
                            else:
                                vs = vst.tile([128, 490], BF, name="vs", tag="vs")
                                nc.vector.tensor_copy(vs[:], acc[:])
                                dstap = bass.AP(
                                    tensor=v_dram[:].tensor,
                                    offset=(blk * 10 * 4 + cto) * 128 * P,
                                    ap=[[P, 128], [4 * 128 * P, 10], [1, P]])
                                nc.sync.dma_start(out=dstap, in_=vs[:])

            # ---------------- phase 2a: QK^T + mask + exp ----------------
            with (
                tc.tile_pool(name="p2a", bufs=1) as p2a,
                tc.tile_pool(name="ps2", bufs=4, space="PSUM") as ps2,
            ):
                mask_t = p2a.tile([CAP, CAP], F32, name="mask")
                nc.sync.dma_start(out=mask_t[:], in_=mask_d[:])
                mask7 = p2a.tile([70, 7, 70], F32, name="mask7")
                for r in range(7):
                    nc.vector.tensor_copy(mask7[:, r, :], mask_t[:70, :70])
                attf = p2a.tile([70, P, 70], F32, name="attf")
                for pg in range(7):
                    aps = ps2.tile([70, 490], F32, name="aps", tag="aps")
                    for pp in range(7):
                        p = pg * 7 + pp
                        for ct in range(4):
                            lhsT = bass.AP(tensor=k_s[ct].tensor, offset=k_s[ct].offset + p,
                                           ap=[k_s[ct].ap[0], [P, 70]])
                            rhs = bass.AP(tensor=q_s[ct].tensor, offset=q_s[ct].offset + p,
                                          ap=[q_s[ct].ap[0], [P, 70]])
                            nc.tensor.matmul(aps[:, pp * 70:(pp + 1) * 70], lhsT, rhs,
                                             start=(ct == 0), stop=(ct == 3))
                    nc.vector.tensor_add(
                        attf[:, pg * 7:(pg + 1) * 7, :].rearrange("a b c -> a (b c)"),
                        aps[:], mask7.rearrange("a b c -> a (b c)"))
                nc.scalar.activation(
                    attw.rearrange("a b c -> a (b c)"),
                    attf.rearrange("a b c -> a (b c)"),
                    func=mybir.ActivationFunctionType.Exp)
            qkp_cm.__exit__(None, None, None)

            # ---------------- phase 2b: rowsum, AV, divide ----------------
            with (
                tc.tile_pool(name="p2b", bufs=1) as p2b,
                tc.tile_pool(name="stats", bufs=1) as stp,
                tc.tile_pool(name="ps3", bufs=4, space="PSUM") as ps3,
                tc.tile_pool(name="ps4", bufs=2, space="PSUM") as ps4,
            ):
                ones_t = p2b.tile([70, 1], BF, name="ones")
                nc.vector.memset(ones_t[:], 1.0)
                rsum = stp.tile([1, NPOS], F32, name="rsum", tag="st")
                for pc in range(7):
                    op = ps4.tile([1, 490], F32, name="op", tag="op")
                    nc.tensor.matmul(
                        op[:], ones_t[:],
                        attw[:, pc * 7:(pc + 1) * 7, :].rearrange("a b c -> a (b c)"),
                        start=True, stop=True)
                    nc.vector.tensor_copy(rsum[:, pc * 490:(pc + 1) * 490], op[:])
                nc.vector.reciprocal(rsum[:], rsum[:])
                nc.sync.dma_start(out=sc1[:], in_=rsum[0:1, :])
                recip_b = p2b.tile([128, NPOS], F32, name="recipb")
                nc.sync.dma_start(out=recip_b[:], in_=bcast_read(sc1, NPOS))
                vth = p2b.tile([70, 2, 128, P], BF, name="vth", tag="vth")
                for h in range(2):
                    if h == 1:
                        vth = p2b.tile([70, 2, 128, P], BF, name="vth2", tag="vth")
                    nc.sync.dma_start(out=vth[:], in_=v_dram[:, h * 2:h * 2 + 2])
                    for ctp in range(2):
                        ct = h * 2 + ctp
                        for pg in range(7):
                            av = ps3.tile([128, 490], F32, name="av", tag="av")
                            for pp in range(7):
                                p = pg * 7 + pp
                                nc.tensor.matmul(av[:, pp * 70:(pp + 1) * 70],
                                                 vth[:, ctp, :, p], attw[:, p, :],
                                                 start=True, stop=True)
                            nc.vector.tensor_copy(
                                virt[ct][:, pg * 7:(pg + 1) * 7, :].rearrange("a b c -> a (b c)"),
                                av[:])
                for ct in range(4):
                    vf = virt[ct].rearrange("a b c -> a (b c)")
                    nc.vector.tensor_mul(vf, vf, recip_b[:])

                # ---- GroupNorm stats (per-i over c,p) ----
                vbfp = p2b.tile([128, NPOS], BF, name="vbf", tag="vbf")
                s1 = stp.tile([1, NPOS], F32, name="s1", tag="st")
                s2 = stp.tile([1, NPOS], F32, name="s2", tag="st2")
                onesf = p2b.tile([128, 1], BF, name="onesf")
                nc.vector.memset(onesf[:], 1.0)
                for which, sdst in ((0, s1), (1, s2)):
                    for chunk in range(7):
                        op = ps4.tile([1, 490], F32, name="op2", tag="op")
                        for ct in range(4):
                            vf = virt[ct].rearrange("a b c -> a (b c)")
                            seg = vf[:, chunk * 490:(chunk + 1) * 490]
                            if which == 0:
                                nc.vector.tensor_copy(vbfp[:, chunk * 490:(chunk + 1) * 490], seg)
                            else:
                                nc.vector.tensor_mul(vbfp[:, chunk * 490:(chunk + 1) * 490], seg, seg)
                            nc.tensor.matmul(op[:], onesf[:],
                                             vbfp[:, chunk * 490:(chunk + 1) * 490],
                                             start=(ct == 0), stop=(ct == 3))
                        nc.vector.tensor_copy(sdst[:, chunk * 490:(chunk + 1) * 490], op[:])

                s1i = p2b.tile([1, 70], F32, name="s1i")
                s2i = p2b.tile([1, 70], F32, name="s2i")
                for src, dsti in ((s1, s1i), (s2, s2i)):
                    v3 = bass.AP(tensor=src.tensor, offset=src.offset,
                                 ap=[src.ap[0], [1, 70], [70, P]])
                    nc.vector.reduce_sum(dsti[:], v3, axis=mybir.AxisListType.X)
                inv_n = 1.0 / (C * P)
                mean_r = p2b.tile([1, 70], F32, name="meanr")
                var_r = p2b.tile([1, 70], F32, name="varr")
                nc.vector.tensor_scalar_mul(mean_r[:], s1i[:], inv_n)
                nc.vector.tensor_scalar_mul(var_r[:], s2i[:], inv_n)
                msq = p2b.tile([1, 70], F32, name="msq")
                nc.vector.tensor_mul(msq[:], mean_r[:], mean_r[:])
                nc.vector.tensor_sub(var_r[:], var_r[:], msq[:])
                eps_t = p2b.tile([1, 1], F32, name="eps")
                nc.vector.memset(eps_t[:], 1e-5)
                nc.scalar.activation(var_r[:], var_r[:],
                                     func=mybir.ActivationFunctionType.Sqrt,
                                     bias=eps_t[:], scale=1.0)
                nc.vector.reciprocal(var_r[:], var_r[:])   # rstd
                negb_r = p2b.tile([1, 70], F32, name="negbr")
                nc.vector.tensor_mul(negb_r[:], mean_r[:], var_r[:])
                nc.vector.tensor_scalar_mul(negb_r[:], negb_r[:], -1.0)
                rstd_f = stp.tile([1, NPOS], F32, name="rstdf", tag="st")
                negb_f = stp.tile([1, NPOS], F32, name="negbf", tag="st2")
                for p in range(P):
                    nc.vector.tensor_copy(rstd_f[:, p * 70:(p + 1) * 70], var_r[:])
                    nc.vector.tensor_copy(negb_f[:, p * 70:(p + 1) * 70], negb_r[:])
                nc.sync.dma_start(out=sc2[:], in_=rstd_f[0:1, :])
                nc.sync.dma_start(out=sc3[:], in_=negb_f[0:1, :])

            # ---------------- phase 3: normalize, relu, out conv, residual --------
            with (
                tc.tile_pool(name="p3", bufs=1) as p3,
                tc.tile_pool(name="wts3", bufs=2) as wts3,
                tc.tile_pool(name="xin3", bufs=3) as xin3,
                tc.tile_pool(name="ost", bufs=3) as ost,
                tc.tile_pool(name="ps5", bufs=8, space="PSUM") as ps5,
            ):
                rstd_b = p3.tile([128, NPOS], F32, name="rstdb")
                negb_b = p3.tile([128, NPOS], F32, name="negbb")
                nc.sync.dma_start(out=rstd_b[:], in_=bcast_read(sc2, NPOS))
                nc.sync.dma_start(out=negb_b[:], in_=bcast_read(sc3, NPOS))
                rp = [p3.tile([128, CAP * 81], BF, name=f"rp{c}") for c in range(4)]
                for ct in range(4):
                    nc.vector.memset(rp[ct][:], 0.0)
                    vf = virt[ct].rearrange("a b c -> a (b c)")
                    nc.vector.tensor_mul(vf, vf, rstd_b[:])
                    nc.vector.tensor_add(vf, vf, negb_b[:])
                    dst = bass.AP(tensor=rp[ct].tensor, offset=rp[ct].offset + 10,
                                  ap=[rp[ct].ap[0], [9, 7], [1, 7], [81, 70]])
                    src = virt[ct].rearrange("a (y x) i -> a y x i", y=7)
                    nc.scalar.activation(dst, src,
                                         func=mybir.ActivationFunctionType.Relu)
                for cto in range(4):
                    wt = wts3.tile([128, 4, 9, 128], BF, name="wt3", tag="wt3")
                    for ci in range(4):
                        srcap = bass.AP(
                            tensor=wo_d[:].tensor, offset=ci * 589824 + cto * 128,
                            ap=[[4608, 128], [512, 9], [1, 128]])
                        nc.sync.dma_start(out=wt[:, ci, :, :], in_=srcap)
                    for blk in range(NB):
                        xit = xin3.tile([128, 490], F32, name="xi", tag="xi")
                        nc.sync.dma_start(
                            out=xit[:], in_=xint_d[cto][:, blk * 490:(blk + 1) * 490])
                        acc = ps5.tile([128, 490], F32, name="acc3", tag="acc3")
                        fi = True
                        for ci in range(4):
                            for tap in range(9):
                                nc.tensor.matmul(acc[:], wt[:, ci, tap, :],
                                                 conv_rhs(rp[ci], blk, tap),
                                                 start=fi, stop=(ci == 3 and tap == 8))
                                fi = False
                        o = ost.tile([128, 490], F32, name="o", tag="o")
                        nc.vector.tensor_add(o[:], acc[:], xit[:])
                        nc.sync.dma_start(
                            out=y_d[cto][:, blk * 490:(blk + 1) * 490], in_=o[:])

    _split_multiwait(nc)
    _NC_CACHE['nc'] = nc
    return nc


def _shard(rois):
    vid = rois[:, 0].astype(np.int64)
    sizes = np.bincount(vid, minlength=32)
    order = np.argsort(-sizes, kind='stable')
    loads = np.zeros(NCORE, np.int64)
    v2c = np.zeros(32, np.int64)
    for v in order:
        c = int(np.argmin(loads))
        loads[c] += sizes[v]
        v2c[v] = c
    core_of_roi = v2c[vid]
    idxs = [np.nonzero(core_of_roi == c)[0] for c in range(NCORE)]
    for ix in idxs:
        assert len(ix) <= CAP, f"core load {len(ix)} exceeds CAP={CAP}"
    return idxs, vid


def kernel(x, rois, w_q, w_k, w_v, w_out, gamma, beta):
    _install_profhook()
    nc = _build()
    x = np.asarray(x, np.float32)
    rois = np.asarray(rois)
    assert np.allclose(np.asarray(gamma), 1.0) and np.allclose(np.asarray(beta), 0.0), \
        "kernel folds GN affine assuming gamma=1, beta=0"
    idxs, vid = _shard(rois)

    def wprep(w, scale=1.0):
        # [co, ci, 1, 3, 3] -> [ci(4,128), tap, co(4,128)] bf16
        a = (np.asarray(w, np.float32)[:, :, 0] * scale).transpose(1, 2, 3, 0)
        return np.ascontiguousarray(
            a.reshape(4, 128, 9, 4, 128)).astype(ml_dtypes.bfloat16)

    wq = wprep(w_q, 1.0 / np.sqrt(np.float32(C)))
    wk, wv, wo = wprep(w_k), wprep(w_v), wprep(w_out)

    in_maps = []
    for c in range(NCORE):
        ix = idxs[c]
        n = len(ix)
        xpad = np.zeros((CAP, C, 9, 9), np.float32)
        xpad[:n, :, 1:8, 1:8] = x[ix, :, 0]
        xp = np.ascontiguousarray(
            xpad.transpose(1, 0, 2, 3).reshape(4, 128, CAP * 81)
        ).astype(ml_dtypes.bfloat16)
        xi = np.zeros((CAP, C, P), np.float32)
        xi[:n] = x[ix, :, 0].reshape(n, C, P)
        xint = np.ascontiguousarray(xi.transpose(1, 0, 2).reshape(4, 128, NPOS))
        ids = np.full(CAP, -1, np.int64)
        ids[:n] = vid[ix]
        ids[n:] = 1000 + np.arange(CAP - n)
        mask = np.where(ids[:, None] == ids[None, :], 0.0, -1e30).astype(np.float32)
        in_maps.append(dict(xp=xp, xint=xint, wq=wq, wk=wk, wv=wv, wo=wo, mask=mask))

    res = run_bass_kernel_spmd(nc, in_maps, list(range(NCORE)))
    kernel.last_exec_ns = res.exec_time_ns

    out = np.empty((512, C, 1, 7, 7), np.float32)
    for c in range(NCORE):
        ix = idxs[c]
        n = len(ix)
        yc = res.results[c]["y"].reshape(C, CAP, P).transpose(1, 0, 2)
        out[ix] = yc[:n].reshape(n, C, 1, 7, 7)
    return out



# revision 11
# speedup vs baseline: 1.3302x; 1.3302x over previous
"""HR2O_NL sparse-attention kernel for 8 Trainium2 NeuronCores.

Sharding: data-parallel over ROI groups (videos exact-cover packed onto 8
cores, whole groups stay local; 64 ROIs/core for the expected input). Conv
weights replicated. Each core: q/k/v 3x3 convs (bf16 matmuls, valid-tap
streaming — no padded positions), per-position masked attention, GroupNorm
(stats in raw-AV space, corrected in row space), relu, out-conv, residual.
"""
import sys, types
import numpy as np
import ml_dtypes

import concourse.bass as bass
import concourse.mybir as mybir
import concourse.tile as tile
from concourse.bass_utils import run_bass_kernel_spmd

BF = mybir.dt.bfloat16
F32 = mybir.dt.float32
C = 512
P = 49            # 7x7 positions
NCORE = 8

TAPS_BASE = [(dy, dx) for dy in (-1, 0, 1) for dx in (-1, 0, 1) if (dy, dx) != (0, 0)]


def _tap_order(ci, nci):
    # full-coverage tap (0,0) carries start (ci==0) and stop (ci==nci-1)
    if ci == nci - 1:
        return TAPS_BASE + [(0, 0)]
    return [(0, 0)] + TAPS_BASE


def _install_profhook():
    if 'antenv.axon_hooks' in sys.modules:
        return
    try:
        from trn_agent_boot.trn_boot import _ntff_profile_via_ctypes
        hook = _ntff_profile_via_ctypes('/opt/axon/libaxon_pjrt.so')
    except Exception:
        hook = None
    m = types.ModuleType('antenv.axon_hooks')
    m.get_axon_ntff_profile_hook = lambda: hook
    sys.modules['antenv.axon_hooks'] = m


def _walk_blocks(bb):
    yield bb
    for inner in getattr(bb, 'blocks', []) or []:
        yield from _walk_blocks(inner)


def _split_multiwait(nc):
    # this walrus build accepts one sync wait per instruction
    fn = nc.m.functions[0]
    for bb in list(_walk_blocks(fn)):
        insts = getattr(bb, 'instructions', None)
        if not insts:
            continue
        new_list, changed = [], False
        for inst in insts:
            si = inst.sync_info
            if si is not None and si.on_wait is not None and len(si.on_wait) > 1:
                waits = list(si.on_wait)
                for j, w in enumerate(waits[:-1]):
                    d = mybir.InstDrain(name=f"{inst.name}_ws{j}", ins=[], outs=[])
                    d.engine = inst.engine
                    d.sync_info = mybir.SyncInfo(on_wait=[w], on_update=[])
                    new_list.append(d)
                si.on_wait = [waits[-1]]
                changed = True
            new_list.append(inst)
        if changed:
            insts[:] = new_list


_NC_CACHE = {}


def _build(cap):
    if cap in _NC_CACHE:
        return _NC_CACHE[cap]
    npos = cap * P
    nfull, rem = divmod(cap, 10)
    blocks = [10] * nfull + ([rem] if rem else [])
    bstart = [sum(blocks[:i]) for i in range(len(blocks))]
    NBK = len(blocks)

    nc = bass.Bass("TRN2", target_bir_lowering=False, debug=False, num_devices=NCORE)
    x_d = nc.dram_tensor("xq", [4, 128, npos], BF, kind="ExternalInput")
    wq_d = nc.dram_tensor("wq", [4, 128, 9, 4, 128], BF, kind="ExternalInput")
    wk_d = nc.dram_tensor("wk", [4, 128, 9, 4, 128], BF, kind="ExternalInput")
    wv_d = nc.dram_tensor("wv", [4, 128, 9, 4, 128], BF, kind="ExternalInput")
    wo_d = nc.dram_tensor("wo", [4, 128, 9, 4, 128], BF, kind="ExternalInput")
    mask_d = nc.dram_tensor("mask", [cap, cap], F32, kind="ExternalInput")
    y_d = nc.dram_tensor("y", [4, 128, npos], F32, kind="ExternalOutput")
    v_dram = nc.dram_tensor("v_sc", [cap, 4, 128, P], BF)

    def conv_views(xt_like, acc_like, blk, dy, dx):
        nb = blocks[blk]
        vy, vx = 7 - abs(dy), 7 - abs(dx)
        oy, ox = max(-dy, 0), max(-dx, 0)
        iy, ix = max(dy, 0), max(dx, 0)
        out_ap = bass.AP(tensor=acc_like.tensor,
                         offset=acc_like.offset + oy * 7 + ox,
                         ap=[acc_like.ap[0], [49, nb], [7, vy], [1, vx]])
        rhs_ap = bass.AP(tensor=xt_like.tensor,
                         offset=xt_like.offset + bstart[blk] * 49 + iy * 7 + ix,
                         ap=[xt_like.ap[0], [49, nb], [7, vy], [1, vx]])
        return out_ap, rhs_ap

    with tile.TileContext(nc) as tc:
        with tc.tile_pool(name="persist", bufs=1) as pp:
            xt = [pp.tile([128, npos], BF, name=f"xt{c}") for c in range(4)]
            attw = pp.tile([cap, P, cap], BF, name="attw")
            virt = [pp.tile([128, npos], BF, name=f"virt{t}") for t in range(4)]
            rsum = pp.tile([1, npos], F32, name="rsum")
            alpha = pp.tile([1, npos], BF, name="alpha")
            beta_t = pp.tile([1, npos], BF, name="beta_t")
            ones1 = pp.tile([1, 128], BF, name="ones1")
            nc.vector.memset(ones1[:], 1.0)

            for c in range(4):
                nc.sync.dma_start(out=xt[c][:], in_=x_d[c])

            vp0_cm = tc.tile_pool(name="vp0", bufs=1)
            vp0 = vp0_cm.__enter__()
            vth0 = vp0.tile([cap, 2, 128, P], BF, name="vth0")

            qkp_cm = tc.tile_pool(name="qk", bufs=1)
            qkp = qkp_cm.__enter__()
            q_s = [qkp.tile([128, npos], BF, name=f"q{t}") for t in range(4)]
            k_s = [qkp.tile([128, npos], BF, name=f"k{t}") for t in range(4)]

            # ---------------- phase 1: q,k,v convs ----------------
            with (
                tc.tile_pool(name="wts", bufs=2) as wts,
                tc.tile_pool(name="vst", bufs=3) as vst,
                tc.tile_pool(name="ps1", bufs=4, space="PSUM") as ps1,
            ):
                for wd, dst in ((wq_d, q_s), (wk_d, k_s), (wv_d, None)):
                    for cto in range(4):
                        wt = wts.tile([128, 4, 9, 128], BF, name="wt", tag="wt")
                        for ci in range(4):
                            srcap = bass.AP(
                                tensor=wd[:].tensor, offset=ci * 589824 + cto * 128,
                                ap=[[4608, 128], [512, 9], [1, 128]])
                            nc.sync.dma_start(out=wt[:, ci, :, :], in_=srcap)
                        for blk in range(NBK):
                            ncols = blocks[blk] * 49
                            acc = ps1.tile([128, 490], F32, name="acc", tag="acc")
                            for ci in range(4):
                                order = _tap_order(ci, 4)
                                for ti, (dy, dx) in enumerate(order):
                                    oap, rap = conv_views(xt[ci], acc, blk, dy, dx)
                                    nc.tensor.matmul(
                                        oap, wt[:, ci, (dy + 1) * 3 + (dx + 1), :], rap,
                                        start=(ci == 0 and ti == 0),
                                        stop=(ci == 3 and ti == 8))
                            cslice = slice(bstart[blk] * 49, bstart[blk] * 49 + ncols)
                            if dst is not None:
                                nc.vector.tensor_copy(dst[cto][:, cslice],
                                                      acc[:, :ncols])
                            else:
                                vs = vst.tile([128, 490], BF, name="vs", tag="vs")
                                nc.scalar.activation(
                                    vs[:, :ncols], acc[:, :ncols],
                                    func=mybir.ActivationFunctionType.Copy)
                                dstap = bass.AP(
                                    tensor=v_dram[:].tensor,
                                    offset=(bstart[blk] * 4 + cto) * 128 * P,
                                    ap=[[P, 128], [4 * 128 * P, blocks[blk]], [1, P]])
                                nc.sync.dma_start(out=dstap, in_=vs[:, :ncols])

            # ---------------- phase 2a: QK^T + mask + exp + rowsum ----------
            nc.sync.dma_start(out=vth0[:], in_=v_dram[:, 0:2])
            with (
                tc.tile_pool(name="p2a", bufs=1) as p2a,
                tc.tile_pool(name="ps2", bufs=2, space="PSUM") as ps2,
                tc.tile_pool(name="ps2b", bufs=2, space="PSUM") as ps2b,
            ):
                mask_t = p2a.tile([cap, cap], F32, name="mask")
                nc.sync.dma_start(out=mask_t[:], in_=mask_d[:])
                mask7 = p2a.tile([cap, 7, cap], F32, name="mask7")
                for r in range(7):
                    nc.vector.tensor_copy(mask7[:, r, :], mask_t[:])
                ones_c = p2a.tile([cap, 1], BF, name="onesc")
                nc.vector.memset(ones_c[:], 1.0)

                def rowsum_pg(pg):
                    op = ps2b.tile([1, 7 * cap], F32, name="op", tag="op")
                    nc.tensor.matmul(
                        op[:], ones_c[:],
                        attw[:, pg * 7:(pg + 1) * 7, :].rearrange("a b c -> a (b c)"),
                        start=True, stop=True)
                    nc.vector.tensor_copy(rsum[:, pg * 7 * cap:(pg + 1) * 7 * cap],
                                          op[:])

                for pg in range(7):
                    aps = ps2.tile([cap, 7 * cap], F32, name="aps", tag="aps")
                    for ppi in range(7):
                        p = pg * 7 + ppi
                        for ct in range(4):
                            lhsT = bass.AP(tensor=k_s[ct].tensor,
                                           offset=k_s[ct].offset + p,
                                           ap=[k_s[ct].ap[0], [P, cap]])
                            rhs = bass.AP(tensor=q_s[ct].tensor,
                                          offset=q_s[ct].offset + p,
                                          ap=[q_s[ct].ap[0], [P, cap]])
                            nc.tensor.matmul(aps[:, ppi * cap:(ppi + 1) * cap],
                                             lhsT, rhs,
                                             start=(ct == 0), stop=(ct == 3))
                    nc.vector.tensor_add(aps[:], aps[:],
                                         mask7.rearrange("a b c -> a (b c)"))
                    nc.scalar.activation(
                        attw[:, pg * 7:(pg + 1) * 7, :].rearrange("a b c -> a (b c)"),
                        aps[:], func=mybir.ActivationFunctionType.Exp)
                    if pg >= 1:
                        rowsum_pg(pg - 1)    # PE consumes previous group's exp
                rowsum_pg(6)
                nc.vector.reciprocal(rsum[:], rsum[:])
            qkp_cm.__exit__(None, None, None)

            # ---------------- phase 2b: AV + GN stats (raw space) ----------
            vp1_cm = tc.tile_pool(name="vp1", bufs=1)
            vp1 = vp1_cm.__enter__()
            vth1 = vp1.tile([cap, 2, 128, P], BF, name="vth1")
            nc.sync.dma_start(out=vth1[:], in_=v_dram[:, 2:4])
            with (
                tc.tile_pool(name="sqp", bufs=2) as sqp,
                tc.tile_pool(name="rowp", bufs=1) as rowp,
                tc.tile_pool(name="ps3", bufs=4, space="PSUM") as ps3,
                tc.tile_pool(name="ps4", bufs=2, space="PSUM") as ps4,
            ):
                s1row = rowp.tile([1, npos], F32, name="s1row")
                s2row = rowp.tile([1, npos], F32, name="s2row")
                onesf = rowp.tile([128, 1], BF, name="onesf")
                nc.vector.memset(onesf[:], 1.0)

                def stats_pg(pg):
                    pslice = slice(pg * 7 * cap, (pg + 1) * 7 * cap)
                    s1ps = ps4.tile([1, 7 * cap], F32, name="s1ps", tag="s1ps")
                    s2ps = ps4.tile([1, 7 * cap], F32, name="s2ps", tag="s2ps")
                    for ct in range(4):
                        nc.tensor.matmul(s1ps[:], onesf[:], virt[ct][:, pslice],
                                         start=(ct == 0), stop=(ct == 3))
                    for ct in range(4):
                        sq = sqp.tile([128, 7 * cap], BF, name="sq", tag="sq")
                        nc.scalar.activation(sq[:], virt[ct][:, pslice],
                                             func=mybir.ActivationFunctionType.Square)
                        nc.tensor.matmul(s2ps[:], onesf[:], sq[:],
                                         start=(ct == 0), stop=(ct == 3))
                    nc.vector.tensor_copy(s1row[:, pslice], s1ps[:])
                    nc.vector.tensor_copy(s2row[:, pslice], s2ps[:])

                for pg in range(7):
                    pslice = slice(pg * 7 * cap, (pg + 1) * 7 * cap)
                    for ct in range(4):
                        vth_h = vth0 if ct < 2 else vth1
                        av = ps3.tile([128, 7 * cap], F32, name="av", tag="av")
                        for ppi in range(7):
                            p = pg * 7 + ppi
                            lhsT = bass.AP(
                                tensor=vth_h.tensor,
                                offset=vth_h.offset + (ct % 2) * 128 * P + p,
                                ap=[vth_h.ap[0], [P, 128]])
                            nc.tensor.matmul(av[:, ppi * cap:(ppi + 1) * cap],
                                             lhsT, attw[:, p, :],
                                             start=True, stop=True)
                        nc.vector.tensor_copy(virt[ct][:, pslice], av[:])
                    if pg >= 1:
                        stats_pg(pg - 1)    # PE consumes previous group's drains
                stats_pg(6)

                # row-space GN math: true s1 = s1row*rsum ; s2 = s2row*rsum^2
                nc.vector.tensor_mul(s1row[:], s1row[:], rsum[:])
                nc.vector.tensor_mul(s2row[:], s2row[:], rsum[:])
                nc.vector.tensor_mul(s2row[:], s2row[:], rsum[:])
                s1i = rowp.tile([1, cap], F32, name="s1i")
                s2i = rowp.tile([1, cap], F32, name="s2i")
                for src, dsti in ((s1row, s1i), (s2row, s2i)):
                    v3 = bass.AP(tensor=src.tensor, offset=src.offset,
                                 ap=[src.ap[0], [1, cap], [cap, P]])
                    nc.vector.reduce_sum(dsti[:], v3, axis=mybir.AxisListType.X)
                inv_n = 1.0 / (C * P)
                mean_r = rowp.tile([1, cap], F32, name="meanr")
                var_r = rowp.tile([1, cap], F32, name="varr")
                nc.vector.tensor_scalar_mul(mean_r[:], s1i[:], inv_n)
                nc.vector.tensor_scalar_mul(var_r[:], s2i[:], inv_n)
                msq = rowp.tile([1, cap], F32, name="msq")
                nc.vector.tensor_mul(msq[:], mean_r[:], mean_r[:])
                nc.vector.tensor_sub(var_r[:], var_r[:], msq[:])
                eps_t = rowp.tile([1, 1], F32, name="eps")
                nc.vector.memset(eps_t[:], 1e-5)
                nc.scalar.activation(var_r[:], var_r[:],
                                     func=mybir.ActivationFunctionType.Sqrt,
                                     bias=eps_t[:], scale=1.0)
                nc.vector.reciprocal(var_r[:], var_r[:])   # rstd per i
                negb_r = rowp.tile([1, cap], F32, name="negbr")
                nc.vector.tensor_mul(negb_r[:], mean_r[:], var_r[:])
                nc.vector.tensor_scalar_mul(negb_r[:], negb_r[:], -1.0)
                # alpha[(p,i)] = rsum_recip * rstd[i] ; beta[(p,i)] = -mu*rstd
                # (s1row reused as f32 staging; its reduction is already done)
                for p in range(P):
                    nc.vector.tensor_copy(s1row[:, p * cap:(p + 1) * cap], var_r[:])
                    nc.vector.tensor_copy(beta_t[:, p * cap:(p + 1) * cap], negb_r[:])
                nc.vector.tensor_mul(s1row[:], s1row[:], rsum[:])
                nc.vector.tensor_copy(alpha[:], s1row[:])
            vp1_cm.__exit__(None, None, None)
            vp0_cm.__exit__(None, None, None)

            # ------- phase 3: per blk: normalize+relu then out conv+residual ----
            with (
                tc.tile_pool(name="rpp", bufs=1) as rpp,
                tc.tile_pool(name="tmp3", bufs=3) as tmp3,
                tc.tile_pool(name="ost", bufs=3) as ost,
                tc.tile_pool(name="ps5", bufs=4, space="PSUM") as ps5,
                tc.tile_pool(name="ps6", bufs=4, space="PSUM") as ps6,
            ):
                rp = [rpp.tile([128, npos], BF, name=f"rp{c}") for c in range(4)]
                wt3 = rpp.tile([128, 4, 4, 9, 128], BF, name="wt3")
                for cto in range(4):
                    for ci in range(4):
                        srcap = bass.AP(
                            tensor=wo_d[:].tensor, offset=ci * 589824 + cto * 128,
                            ap=[[4608, 128], [512, 9], [1, 128]])
                        nc.sync.dma_start(out=wt3[:, cto, ci, :, :], in_=srcap)
                for blk in range(NBK):
                    nb = blocks[blk]
                    ncols = nb * 49
                    cslice = slice(bstart[blk] * 49, bstart[blk] * 49 + ncols)

                    def rowview(t):
                        return bass.AP(tensor=t.tensor,
                                       offset=t.offset + bstart[blk],
                                       ap=[t.ap[0], [1, nb], [cap, P]])
                    a_ps = ps6.tile([128, 490], F32, name="a_ps", tag="abps")
                    b_ps = ps6.tile([128, 490], F32, name="b_ps", tag="abps")
                    nc.tensor.matmul(a_ps[:, :ncols], ones1[:], rowview(alpha),
                                     start=True, stop=True)
                    nc.tensor.matmul(b_ps[:, :ncols], ones1[:], rowview(beta_t),
                                     start=True, stop=True)
                    for ct in range(4):
                        vview = bass.AP(tensor=virt[ct].tensor,
                                        offset=virt[ct].offset + bstart[blk],
                                        ap=[virt[ct].ap[0], [1, nb], [cap, P]])
                        t1 = tmp3.tile([128, 490], F32, name="t1", tag="t1")
                        nc.vector.tensor_mul(
                            t1[:, :ncols].rearrange("a (b c) -> a b c", b=nb),
                            vview, a_ps[:, :ncols].rearrange("a (b c) -> a b c", b=nb))
                        nc.vector.tensor_add(t1[:, :ncols], t1[:, :ncols],
                                             b_ps[:, :ncols])
                        nc.scalar.activation(rp[ct][:, cslice], t1[:, :ncols],
                                             func=mybir.ActivationFunctionType.Relu)
                    for cto in range(4):
                        acc = ps5.tile([128, 490], F32, name="acc3", tag="acc3")
                        for ci in range(4):
                            order = _tap_order(ci, 4)
                            for ti, (dy, dx) in enumerate(order):
                                oap, rap = conv_views(rp[ci], acc, blk, dy, dx)
                                nc.tensor.matmul(
                                    oap, wt3[:, cto, ci, (dy + 1) * 3 + (dx + 1), :],
                                    rap,
                                    start=(ci == 0 and ti == 0),
                                    stop=(ci == 3 and ti == 8))
                        o = ost.tile([128, 490], F32, name="o", tag="o")
                        nc.vector.tensor_add(o[:, :ncols], acc[:, :ncols],
                                             xt[cto][:, cslice])
                        nc.sync.dma_start(out=y_d[cto][:, cslice], in_=o[:, :ncols])

    _split_multiwait(nc)
    _NC_CACHE[cap] = (nc, blocks)
    return _NC_CACHE[cap]


def _find_subset(avail, target):
    items = sorted(avail, key=lambda t: -t[0])
    suffix = [0] * (len(items) + 1)
    for i in range(len(items) - 1, -1, -1):
        suffix[i] = suffix[i + 1] + items[i][0]

    def dfs(i, rem, chosen):
        if rem == 0:
            return list(chosen)
        if i >= len(items) or rem < 0 or suffix[i] < rem:
            return None
        r = dfs(i + 1, rem - items[i][0], chosen + [items[i]])
        if r:
            return r
        return dfs(i + 1, rem, chosen)

    return dfs(0, target, [])


def _shard(rois):
    vid = rois[:, 0].astype(np.int64)
    sizes = np.bincount(vid, minlength=int(vid.max()) + 1)
    nvid = len(sizes)
    total = int(sizes.sum())
    per = total // NCORE
    v2c = None
    if total % NCORE == 0:
        avail = [(int(s), i) for i, s in enumerate(sizes) if s > 0]
        assign = {}
        ok = True
        work = list(avail)
        for b in range(NCORE - 1):
            sub = _find_subset(work, per)
            if sub is None:
                ok = False
                break
            for t in sub:
                assign[t[1]] = b
                work.remove(t)
        if ok:
            for t in work:
                assign[t[1]] = NCORE - 1
            v2c = np.zeros(nvid, np.int64)
            for v, c in assign.items():
                v2c[v] = c
            cap = per
    if v2c is None:
        order = np.argsort(-sizes, kind='stable')
        loads = np.zeros(NCORE, np.int64)
        v2c = np.zeros(nvid, np.int64)
        for v in order:
            if sizes[v] == 0:
                continue
            c = int(np.argmin(loads))
            loads[c] += sizes[v]
            v2c[v] = c
        cap = int(loads.max())
    core_of_roi = v2c[vid]
    idxs = [np.nonzero(core_of_roi == c)[0] for c in range(NCORE)]
    return idxs, vid, cap


def kernel(x, rois, w_q, w_k, w_v, w_out, gamma, beta):
    _install_profhook()
    x = np.asarray(x, np.float32)
    rois = np.asarray(rois)
    assert np.allclose(np.asarray(gamma), 1.0) and np.allclose(np.asarray(beta), 0.0), \
        "kernel folds GN affine assuming gamma=1, beta=0"
    idxs, vid, cap = _shard(rois)
    nc, blocks = _build(cap)
    npos = cap * P

    def wprep(w, scale=1.0):
        # [co, ci, 1, 3, 3] -> [ci(4,128), tap, co(4,128)] bf16
        a = (np.asarray(w, np.float32)[:, :, 0] * scale).transpose(1, 2, 3, 0)
        return np.ascontiguousarray(
            a.reshape(4, 128, 9, 4, 128)).astype(ml_dtypes.bfloat16)

    wq = wprep(w_q, 1.0 / np.sqrt(np.float32(C)))
    wk, wv, wo = wprep(w_k), wprep(w_v), wprep(w_out)

    in_maps = []
    for c in range(NCORE):
        ix = idxs[c]
        n = len(ix)
        xi = np.zeros((cap, C, P), np.float32)
        xi[:n] = x[ix, :, 0].reshape(n, C, P)
        xq = np.ascontiguousarray(
            xi.transpose(1, 0, 2).reshape(4, 128, npos)).astype(ml_dtypes.bfloat16)
        ids = np.full(cap, -1, np.int64)
        ids[:n] = vid[ix]
        ids[n:] = 10 ** 6 + np.arange(cap - n)
        mask = np.where(ids[:, None] == ids[None, :], 0.0, -1e30).astype(np.float32)
        in_maps.append(dict(xq=xq, wq=wq, wk=wk, wv=wv, wo=wo, mask=mask))

    res = run_bass_kernel_spmd(nc, in_maps, list(range(NCORE)))
    kernel.last_exec_ns = res.exec_time_ns

    out = np.empty((512, C, 1, 7, 7), np.float32)
    for c in range(NCORE):
        ix = idxs[c]
        n = len(ix)
        yc = res.results[c]["y"].reshape(C, cap, P).transpose(1, 0, 2)
        out[ix] = yc[:n].reshape(n, C, 1, 7, 7)
    return out


# revision 15
# speedup vs baseline: 1.3850x; 1.0412x over previous
"""HR2O_NL sparse-attention kernel for 8 Trainium2 NeuronCores.

Sharding: data-parallel over ROI groups (videos exact-cover packed onto 8
cores, whole groups stay local; 64 ROIs/core for the expected input). Conv
weights replicated. Each core: q/k/v 3x3 convs (bf16 matmuls, valid-tap
streaming — no padded positions), per-position masked attention, GroupNorm
(stats in raw-AV space, corrected in row space), relu, out-conv, residual.
"""
import sys, types
import numpy as np
import ml_dtypes

import concourse.bass as bass
import concourse.mybir as mybir
import concourse.tile as tile
from concourse.bass_utils import run_bass_kernel_spmd

BF = mybir.dt.bfloat16
F32 = mybir.dt.float32
C = 512
P = 49            # 7x7 positions
NCORE = 8

TAPS_BASE = [(dy, dx) for dy in (-1, 0, 1) for dx in (-1, 0, 1) if (dy, dx) != (0, 0)]


def _tap_order(ci, nci):
    # full-coverage tap (0,0) carries start (ci==0) and stop (ci==nci-1)
    if ci == nci - 1:
        return TAPS_BASE + [(0, 0)]
    return [(0, 0)] + TAPS_BASE


def _install_profhook():
    if 'antenv.axon_hooks' in sys.modules:
        return
    try:
        from trn_agent_boot.trn_boot import _ntff_profile_via_ctypes
        hook = _ntff_profile_via_ctypes('/opt/axon/libaxon_pjrt.so')
    except Exception:
        hook = None
    m = types.ModuleType('antenv.axon_hooks')
    m.get_axon_ntff_profile_hook = lambda: hook
    sys.modules['antenv.axon_hooks'] = m


def _walk_blocks(bb):
    yield bb
    for inner in getattr(bb, 'blocks', []) or []:
        yield from _walk_blocks(inner)


def _split_multiwait(nc):
    # this walrus build accepts one sync wait per instruction
    fn = nc.m.functions[0]
    for bb in list(_walk_blocks(fn)):
        insts = getattr(bb, 'instructions', None)
        if not insts:
            continue
        new_list, changed = [], False
        for inst in insts:
            si = inst.sync_info
            if si is not None and si.on_wait is not None and len(si.on_wait) > 1:
                waits = list(si.on_wait)
                for j, w in enumerate(waits[:-1]):
                    d = mybir.InstDrain(name=f"{inst.name}_ws{j}", ins=[], outs=[])
                    d.engine = inst.engine
                    d.sync_info = mybir.SyncInfo(on_wait=[w], on_update=[])
                    new_list.append(d)
                si.on_wait = [waits[-1]]
                changed = True
            new_list.append(inst)
        if changed:
            insts[:] = new_list


_NC_CACHE = {}


def _build(cap):
    if cap in _NC_CACHE:
        return _NC_CACHE[cap]
    npos = cap * P
    nfull, rem = divmod(cap, 10)
    blocks = [10] * nfull + ([rem] if rem else [])
    bstart = [sum(blocks[:i]) for i in range(len(blocks))]
    NBK = len(blocks)

    nc = bass.Bass("TRN2", target_bir_lowering=False, debug=False, num_devices=NCORE)
    x_d = nc.dram_tensor("xq", [4, 128, npos], BF, kind="ExternalInput")
    wq_d = nc.dram_tensor("wq", [4, 128, 9, 4, 128], BF, kind="ExternalInput")
    wk_d = nc.dram_tensor("wk", [4, 128, 9, 4, 128], BF, kind="ExternalInput")
    wv_d = nc.dram_tensor("wv", [4, 128, 9, 4, 128], BF, kind="ExternalInput")
    wo_d = nc.dram_tensor("wo", [4, 128, 9, 4, 128], BF, kind="ExternalInput")
    mask_d = nc.dram_tensor("mask", [cap, cap], F32, kind="ExternalInput")
    y_d = nc.dram_tensor("y", [4, 128, npos], F32, kind="ExternalOutput")
    v_dram = nc.dram_tensor("v_sc", [cap, 4, 128, P], BF)

    def conv_views(xt_like, acc_like, blk, dy, dx):
        nb = blocks[blk]
        vy, vx = 7 - abs(dy), 7 - abs(dx)
        oy, ox = max(-dy, 0), max(-dx, 0)
        iy, ix = max(dy, 0), max(dx, 0)
        out_ap = bass.AP(tensor=acc_like.tensor,
                         offset=acc_like.offset + oy * 7 + ox,
                         ap=[acc_like.ap[0], [49, nb], [7, vy], [1, vx]])
        rhs_ap = bass.AP(tensor=xt_like.tensor,
                         offset=xt_like.offset + bstart[blk] * 49 + iy * 7 + ix,
                         ap=[xt_like.ap[0], [49, nb], [7, vy], [1, vx]])
        return out_ap, rhs_ap

    with tile.TileContext(nc) as tc:
        with tc.tile_pool(name="persist", bufs=1) as pp:
            xt = [pp.tile([128, npos], BF, name=f"xt{c}") for c in range(4)]
            attw = pp.tile([cap, P, cap], BF, name="attw")
            virt = [pp.tile([128, npos], BF, name=f"virt{t}") for t in range(4)]
            rsum = pp.tile([1, npos], F32, name="rsum")
            alpha = pp.tile([1, npos], BF, name="alpha")
            beta_t = pp.tile([1, npos], BF, name="beta_t")
            ones1 = pp.tile([1, 128], BF, name="ones1")
            nc.vector.memset(ones1[:], 1.0)

            for c in range(4):
                nc.sync.dma_start(out=xt[c][:], in_=x_d[c])

            vp0_cm = tc.tile_pool(name="vp0", bufs=1)
            vp0 = vp0_cm.__enter__()
            vth0 = vp0.tile([cap, 2, 128, P], BF, name="vth0")

            qkp_cm = tc.tile_pool(name="qk", bufs=1)
            qkp = qkp_cm.__enter__()
            q_s = [qkp.tile([128, npos], BF, name=f"q{t}") for t in range(4)]
            k_s = [qkp.tile([128, npos], BF, name=f"k{t}") for t in range(4)]

            # ---------------- phase 1: q,k,v convs ----------------
            with (
                tc.tile_pool(name="wts", bufs=2) as wts,
                tc.tile_pool(name="vst", bufs=3) as vst,
                tc.tile_pool(name="ps1", bufs=4, space="PSUM") as ps1,
            ):
                for wd, dst in ((wq_d, q_s), (wk_d, k_s), (wv_d, None)):
                    for cto in range(4):
                        wt = wts.tile([128, 4, 9, 128], BF, name="wt", tag="wt")
                        for ci in range(4):
                            srcap = bass.AP(
                                tensor=wd[:].tensor, offset=ci * 589824 + cto * 128,
                                ap=[[4608, 128], [512, 9], [1, 128]])
                            nc.sync.dma_start(out=wt[:, ci, :, :], in_=srcap)
                        for blk in range(NBK):
                            ncols = blocks[blk] * 49
                            acc = ps1.tile([128, 490], F32, name="acc", tag="acc")
                            for ci in range(4):
                                order = _tap_order(ci, 4)
                                for ti, (dy, dx) in enumerate(order):
                                    oap, rap = conv_views(xt[ci], acc, blk, dy, dx)
                                    nc.tensor.matmul(
                                        oap, wt[:, ci, (dy + 1) * 3 + (dx + 1), :], rap,
                                        start=(ci == 0 and ti == 0),
                                        stop=(ci == 3 and ti == 8))
                            cslice = slice(bstart[blk] * 49, bstart[blk] * 49 + ncols)
                            if dst is not None:
                                nc.vector.tensor_copy(dst[cto][:, cslice],
                                                      acc[:, :ncols])
                            else:
                                vs = vst.tile([128, 490], BF, name="vs", tag="vs")
                                nc.scalar.activation(
                                    vs[:, :ncols], acc[:, :ncols],
                                    func=mybir.ActivationFunctionType.Copy)
                                dstap = bass.AP(
                                    tensor=v_dram[:].tensor,
                                    offset=(bstart[blk] * 4 + cto) * 128 * P,
                                    ap=[[P, 128], [4 * 128 * P, blocks[blk]], [1, P]])
                                nc.sync.dma_start(out=dstap, in_=vs[:, :ncols])

            # ---------------- phase 2a: QK^T + mask + exp + rowsum ----------
            nc.sync.dma_start(out=vth0[:], in_=v_dram[:, 0:2])
            with (
                tc.tile_pool(name="p2a", bufs=1) as p2a,
                tc.tile_pool(name="ps2", bufs=2, space="PSUM") as ps2,
                tc.tile_pool(name="ps2b", bufs=2, space="PSUM") as ps2b,
            ):
                mask_t = p2a.tile([cap, cap], F32, name="mask")
                nc.sync.dma_start(out=mask_t[:], in_=mask_d[:])
                mask7 = p2a.tile([cap, 7, cap], F32, name="mask7")
                for r in range(7):
                    nc.vector.tensor_copy(mask7[:, r, :], mask_t[:])
                ones_c = p2a.tile([cap, 1], BF, name="onesc")
                nc.vector.memset(ones_c[:], 1.0)

                def rowsum_pg(pg):
                    op = ps2b.tile([1, 7 * cap], F32, name="op", tag="op")
                    nc.tensor.matmul(
                        op[:], ones_c[:],
                        attw[:, pg * 7:(pg + 1) * 7, :].rearrange("a b c -> a (b c)"),
                        start=True, stop=True)
                    nc.vector.tensor_copy(rsum[:, pg * 7 * cap:(pg + 1) * 7 * cap],
                                          op[:])

                for pg in range(7):
                    aps = ps2.tile([cap, 7 * cap], F32, name="aps", tag="aps")
                    for ppi in range(7):
                        p = pg * 7 + ppi
                        for ct in range(4):
                            lhsT = bass.AP(tensor=k_s[ct].tensor,
                                           offset=k_s[ct].offset + p,
                                           ap=[k_s[ct].ap[0], [P, cap]])
                            rhs = bass.AP(tensor=q_s[ct].tensor,
                                          offset=q_s[ct].offset + p,
                                          ap=[q_s[ct].ap[0], [P, cap]])
                            nc.tensor.matmul(aps[:, ppi * cap:(ppi + 1) * cap],
                                             lhsT, rhs,
                                             start=(ct == 0), stop=(ct == 3))
                    nc.vector.tensor_add(aps[:], aps[:],
                                         mask7.rearrange("a b c -> a (b c)"))
                    nc.scalar.activation(
                        attw[:, pg * 7:(pg + 1) * 7, :].rearrange("a b c -> a (b c)"),
                        aps[:], func=mybir.ActivationFunctionType.Exp)
                    if pg >= 1:
                        rowsum_pg(pg - 1)    # PE consumes previous group's exp
                rowsum_pg(6)
            qkp_cm.__exit__(None, None, None)

            # ---------------- phase 2b: AV + GN stats (raw space) ----------
            vp1_cm = tc.tile_pool(name="vp1", bufs=1)
            vp1 = vp1_cm.__enter__()
            vth1 = vp1.tile([cap, 2, 128, P], BF, name="vth1")
            nc.sync.dma_start(out=vth1[:], in_=v_dram[:, 2:4])
            with (
                tc.tile_pool(name="sqp", bufs=2) as sqp,
                tc.tile_pool(name="rowp", bufs=1) as rowp,
                tc.tile_pool(name="ps3", bufs=4, space="PSUM") as ps3,
                tc.tile_pool(name="ps4", bufs=2, space="PSUM") as ps4,
            ):
                s1row = rowp.tile([1, npos], F32, name="s1row")
                s2row = rowp.tile([1, npos], F32, name="s2row")
                onesf = rowp.tile([128, 1], BF, name="onesf")
                nc.vector.memset(onesf[:], 1.0)

                # reciprocal of rowsum in 2D (49 partitions) via DMA bounce —
                # single-partition reciprocal on [1,npos] costs ~20us on DVE
                r2d = rowp.tile([P, cap], F32, name="r2d")
                nc.sync.dma_start(out=r2d[:], in_=rsum[0:1, :])
                nc.vector.reciprocal(r2d[:], r2d[:])
                nc.sync.dma_start(out=rsum[0:1, :], in_=r2d[:])
                recip_bf = rowp.tile([1, npos], BF, name="recip_bf")
                nc.vector.tensor_copy(recip_bf[:], rsum[:])

                def stats_pg(pg):
                    pslice = slice(pg * 7 * cap, (pg + 1) * 7 * cap)
                    s1ps = ps4.tile([1, 7 * cap], F32, name="s1ps", tag="s1ps")
                    s2ps = ps4.tile([1, 7 * cap], F32, name="s2ps", tag="s2ps")
                    for ct in range(4):
                        nc.tensor.matmul(s1ps[:], onesf[:], virt[ct][:, pslice],
                                         start=(ct == 0), stop=(ct == 3))
                    for ct in range(4):
                        sq = sqp.tile([128, 7 * cap], BF, name="sq", tag="sq")
                        nc.scalar.activation(sq[:], virt[ct][:, pslice],
                                             func=mybir.ActivationFunctionType.Square)
                        nc.tensor.matmul(s2ps[:], onesf[:], sq[:],
                                         start=(ct == 0), stop=(ct == 3))
                    # fold the softmax normalization in at drain time
                    nc.vector.tensor_mul(s1row[:, pslice], s1ps[:], rsum[:, pslice])
                    nc.vector.tensor_mul(s2row[:, pslice], s2ps[:], rsum[:, pslice])
                    nc.vector.tensor_mul(s2row[:, pslice], s2row[:, pslice],
                                         rsum[:, pslice])

                for pg in range(7):
                    pslice = slice(pg * 7 * cap, (pg + 1) * 7 * cap)
                    for ct in range(4):
                        vth_h = vth0 if ct < 2 else vth1
                        av = ps3.tile([128, 7 * cap], F32, name="av", tag="av")
                        for ppi in range(7):
                            p = pg * 7 + ppi
                            lhsT = bass.AP(
                                tensor=vth_h.tensor,
                                offset=vth_h.offset + (ct % 2) * 128 * P + p,
                                ap=[vth_h.ap[0], [P, 128]])
                            nc.tensor.matmul(av[:, ppi * cap:(ppi + 1) * cap],
                                             lhsT, attw[:, p, :],
                                             start=True, stop=True)
                        nc.vector.tensor_copy(virt[ct][:, pslice], av[:])
                    if pg >= 1:
                        stats_pg(pg - 1)    # PE consumes previous group's drains
                stats_pg(6)

                # row-space GN math (normalization already folded in at drain)
                s1i = rowp.tile([1, cap], F32, name="s1i")
                s2i = rowp.tile([1, cap], F32, name="s2i")
                for src, dsti in ((s1row, s1i), (s2row, s2i)):
                    v3 = bass.AP(tensor=src.tensor, offset=src.offset,
                                 ap=[src.ap[0], [1, cap], [cap, P]])
                    nc.vector.reduce_sum(dsti[:], v3, axis=mybir.AxisListType.X)
                inv_n = 1.0 / (C * P)
                mean_r = rowp.tile([1, cap], F32, name="meanr")
                var_r = rowp.tile([1, cap], F32, name="varr")
                nc.vector.tensor_scalar_mul(mean_r[:], s1i[:], inv_n)
                nc.vector.tensor_scalar_mul(var_r[:], s2i[:], inv_n)
                msq = rowp.tile([1, cap], F32, name="msq")
                nc.vector.tensor_mul(msq[:], mean_r[:], mean_r[:])
                nc.vector.tensor_sub(var_r[:], var_r[:], msq[:])
                eps_t = rowp.tile([1, 1], F32, name="eps")
                nc.vector.memset(eps_t[:], 1e-5)
                nc.scalar.activation(var_r[:], var_r[:],
                                     func=mybir.ActivationFunctionType.Sqrt,
                                     bias=eps_t[:], scale=1.0)
                nc.vector.reciprocal(var_r[:], var_r[:])   # rstd per i
                negb_r = rowp.tile([1, cap], F32, name="negbr")
                nc.vector.tensor_mul(negb_r[:], mean_r[:], var_r[:])
                nc.vector.tensor_scalar_mul(negb_r[:], negb_r[:], -1.0)
                # alpha[(p,i)] = rsum_recip * rstd[i] ; beta[(p,i)] = -mu*rstd
                # stride-0 broadcast views replicate the [1,cap] rows over p
                var_b = rowp.tile([1, cap], BF, name="var_b")
                negb_b = rowp.tile([1, cap], BF, name="negb_b")
                nc.vector.tensor_copy(var_b[:], var_r[:])
                nc.vector.tensor_copy(negb_b[:], negb_r[:])

                def rep_view(t):
                    return bass.AP(tensor=t.tensor, offset=t.offset,
                                   ap=[t.ap[0], [0, P], [1, cap]])
                nc.vector.tensor_mul(
                    alpha.rearrange("a (b c) -> a b c", b=P),
                    rep_view(var_b),
                    recip_bf.rearrange("a (b c) -> a b c", b=P))
                nc.vector.tensor_copy(
                    beta_t.rearrange("a (b c) -> a b c", b=P), rep_view(negb_b))
            vp1_cm.__exit__(None, None, None)
            vp0_cm.__exit__(None, None, None)

            # ------- phase 3: per blk: normalize+relu then out conv+residual ----
            with (
                tc.tile_pool(name="rpp", bufs=1) as rpp,
                tc.tile_pool(name="tmp3", bufs=3) as tmp3,
                tc.tile_pool(name="ost", bufs=3) as ost,
                tc.tile_pool(name="ps5", bufs=4, space="PSUM") as ps5,
                tc.tile_pool(name="ps6", bufs=4, space="PSUM") as ps6,
            ):
                rp = [rpp.tile([128, npos], BF, name=f"rp{c}") for c in range(4)]
                wt3 = rpp.tile([128, 4, 4, 9, 128], BF, name="wt3")
                for cto in range(4):
                    for ci in range(4):
                        srcap = bass.AP(
                            tensor=wo_d[:].tensor, offset=ci * 589824 + cto * 128,
                            ap=[[4608, 128], [512, 9], [1, 128]])
                        nc.sync.dma_start(out=wt3[:, cto, ci, :, :], in_=srcap)
                for blk in range(NBK):
                    nb = blocks[blk]
                    ncols = nb * 49
                    cslice = slice(bstart[blk] * 49, bstart[blk] * 49 + ncols)

                    def rowview(t):
                        return bass.AP(tensor=t.tensor,
                                       offset=t.offset + bstart[blk],
                                       ap=[t.ap[0], [1, nb], [cap, P]])
                    a_ps = ps6.tile([128, 490], F32, name="a_ps", tag="abps")
                    b_ps = ps6.tile([128, 490], F32, name="b_ps", tag="abps")
                    nc.tensor.matmul(a_ps[:, :ncols], ones1[:], rowview(alpha),
                                     start=True, stop=True)
                    nc.tensor.matmul(b_ps[:, :ncols], ones1[:], rowview(beta_t),
                                     start=True, stop=True)
                    for ct in range(4):
                        vview = bass.AP(tensor=virt[ct].tensor,
                                        offset=virt[ct].offset + bstart[blk],
                                        ap=[virt[ct].ap[0], [1, nb], [cap, P]])
                        t1 = tmp3.tile([128, 490], F32, name="t1", tag="t1")
                        nc.vector.tensor_mul(
                            t1[:, :ncols].rearrange("a (b c) -> a b c", b=nb),
                            vview, a_ps[:, :ncols].rearrange("a (b c) -> a b c", b=nb))
                        nc.vector.tensor_add(t1[:, :ncols], t1[:, :ncols],
                                             b_ps[:, :ncols])
                        nc.scalar.activation(rp[ct][:, cslice], t1[:, :ncols],
                                             func=mybir.ActivationFunctionType.Relu)
                    for cto in range(4):
                        acc = ps5.tile([128, 490], F32, name="acc3", tag="acc3")
                        for ci in range(4):
                            order = _tap_order(ci, 4)
                            for ti, (dy, dx) in enumerate(order):
                                oap, rap = conv_views(rp[ci], acc, blk, dy, dx)
                                nc.tensor.matmul(
                                    oap, wt3[:, cto, ci, (dy + 1) * 3 + (dx + 1), :],
                                    rap,
                                    start=(ci == 0 and ti == 0),
                                    stop=(ci == 3 and ti == 8))
                        o = ost.tile([128, 490], F32, name="o", tag="o")
                        nc.vector.tensor_add(o[:, :ncols], acc[:, :ncols],
                                             xt[cto][:, cslice])
                        nc.sync.dma_start(out=y_d[cto][:, cslice], in_=o[:, :ncols])

    _split_multiwait(nc)
    _NC_CACHE[cap] = (nc, blocks)
    return _NC_CACHE[cap]


def _find_subset(avail, target):
    items = sorted(avail, key=lambda t: -t[0])
    suffix = [0] * (len(items) + 1)
    for i in range(len(items) - 1, -1, -1):
        suffix[i] = suffix[i + 1] + items[i][0]

    def dfs(i, rem, chosen):
        if rem == 0:
            return list(chosen)
        if i >= len(items) or rem < 0 or suffix[i] < rem:
            return None
        r = dfs(i + 1, rem - items[i][0], chosen + [items[i]])
        if r:
            return r
        return dfs(i + 1, rem, chosen)

    return dfs(0, target, [])


def _shard(rois):
    vid = rois[:, 0].astype(np.int64)
    sizes = np.bincount(vid, minlength=int(vid.max()) + 1)
    nvid = len(sizes)
    total = int(sizes.sum())
    per = total // NCORE
    v2c = None
    if total % NCORE == 0:
        avail = [(int(s), i) for i, s in enumerate(sizes) if s > 0]
        assign = {}
        ok = True
        work = list(avail)
        for b in range(NCORE - 1):
            sub = _find_subset(work, per)
            if sub is None:
                ok = False
                break
            for t in sub:
                assign[t[1]] = b
                work.remove(t)
        if ok:
            for t in work:
                assign[t[1]] = NCORE - 1
            v2c = np.zeros(nvid, np.int64)
            for v, c in assign.items():
                v2c[v] = c
            cap = per
    if v2c is None:
        order = np.argsort(-sizes, kind='stable')
        loads = np.zeros(NCORE, np.int64)
        v2c = np.zeros(nvid, np.int64)
        for v in order:
            if sizes[v] == 0:
                continue
            c = int(np.argmin(loads))
            loads[c] += sizes[v]
            v2c[v] = c
        cap = int(loads.max())
    core_of_roi = v2c[vid]
    idxs = [np.nonzero(core_of_roi == c)[0] for c in range(NCORE)]
    return idxs, vid, cap


def kernel(x, rois, w_q, w_k, w_v, w_out, gamma, beta):
    _install_profhook()
    x = np.asarray(x, np.float32)
    rois = np.asarray(rois)
    assert np.allclose(np.asarray(gamma), 1.0) and np.allclose(np.asarray(beta), 0.0), \
        "kernel folds GN affine assuming gamma=1, beta=0"
    idxs, vid, cap = _shard(rois)
    nc, blocks = _build(cap)
    npos = cap * P

    def wprep(w, scale=1.0):
        # [co, ci, 1, 3, 3] -> [ci(4,128), tap, co(4,128)] bf16
        a = (np.asarray(w, np.float32)[:, :, 0] * scale).transpose(1, 2, 3, 0)
        return np.ascontiguousarray(
            a.reshape(4, 128, 9, 4, 128)).astype(ml_dtypes.bfloat16)

    wq = wprep(w_q, 1.0 / np.sqrt(np.float32(C)))
    wk, wv, wo = wprep(w_k), wprep(w_v), wprep(w_out)

    in_maps = []
    for c in range(NCORE):
        ix = idxs[c]
        n = len(ix)
        xi = np.zeros((cap, C, P), np.float32)
        xi[:n] = x[ix, :, 0].reshape(n, C, P)
        xq = np.ascontiguousarray(
            xi.transpose(1, 0, 2).reshape(4, 128, npos)).astype(ml_dtypes.bfloat16)
        ids = np.full(cap, -1, np.int64)
        ids[:n] = vid[ix]
        ids[n:] = 10 ** 6 + np.arange(cap - n)
        mask = np.where(ids[:, None] == ids[None, :], 0.0, -1e30).astype(np.float32)
        in_maps.append(dict(xq=xq, wq=wq, wk=wk, wv=wv, wo=wo, mask=mask))

    res = run_bass_kernel_spmd(nc, in_maps, list(range(NCORE)))
    kernel.last_exec_ns = res.exec_time_ns

    out = np.empty((512, C, 1, 7, 7), np.float32)
    for c in range(NCORE):
        ix = idxs[c]
        n = len(ix)
        yc = res.results[c]["y"].reshape(C, cap, P).transpose(1, 0, 2)
        out[ix] = yc[:n].reshape(n, C, 1, 7, 7)
    return out
